# revision 14
# baseline (speedup 1.0000x reference)
"""MoE (E=64, K=8, D=512, I=1024, C=1024) on 8 TRN2 NeuronCores.

Strategy (expert-parallel, per sharding hint):
  - Host: gating (scores/softmax/top-k), dispatch bookkeeping (stable sort by
    expert, capacity slots) and packing of the per-core dispatch buffers.
    All device tensors are host-packed into their exact SBUF images so every
    DMA is a single large contiguous transfer (8-16KB per partition row).
  - Device (SPMD, 8 cores, 8 experts/core):
      stage1 (x@w1, x@w3): bf16 tokens x fp8-e3m4 weights (w1 scaled by 32,
        descaled for free via the ACT silu scale; w3 scaled by 8 so the DVE
        silu*b product lands directly as 8*h in fp8-e4m3).
      stage2 (h@w2): double-pumped fp8 matmul (DoubleRow, 2 MAC/cycle/PE)
        with h and w2 (scaled by 32) both e4m3.  Output is 256*y in bf16;
        the 1/256 descale is folded into the host-side combine weights.
      shared expert: data-parallel shard of tokens, full bf16 (its output
        dominates the result norm, so it stays high precision).
  - Host: weighted combine of expert outputs back to token order + shared
    expert add.

Quantization error (measured vs fp32 reference on the fixed seed):
  rel_err ~= 0.015 (gate is 2e-2); all-bf16 fallback (FP8=False) is 0.004.

kernel(**inputs) takes the FULL unsharded inputs and returns the FULL
[B, S, D] float32 output.
"""

import sys

for _p in ("/opt/trn_rl_repo",):
    if _p not in sys.path:
        sys.path.append(_p)

import numpy as np
import ml_dtypes

import concourse.bacc as bacc
import concourse.mybir as mybir
import concourse.tile as tile
from concourse.bass_utils import run_bass_kernel_spmd

E = 64          # experts
K = 8           # top-k
D = 512         # model dim
I = 1024        # expert inner dim
CAP = 1024      # per-expert capacity in the reference
NCORES = 8
EL = E // NCORES  # experts per core (8)

BF16 = mybir.dt.bfloat16
F32 = mybir.dt.float32
F8E3 = mybir.dt.float8e3   # e3m4
F8E4 = mybir.dt.float8e4   # e4m3 (TRN variant, max 240)

NP_BF16 = ml_dtypes.bfloat16
NP_E3M4 = ml_dtypes.float8_e3m4
NP_E4M3 = ml_dtypes.float8_e4m3

S1_W1_SCALE = 32.0   # w1 stored as e3m4(32*w1); descaled in ACT silu
S3_SCALE = 8.0       # w3 stored as e3m4(8*w3); makes h_tile = 8*h
W2_SCALE = 32.0      # w2 stored as e4m3(32*w2)
OUT_DESCALE = 1.0 / (S3_SCALE * W2_SCALE)   # folded into host combine

# set by test harness: when True, kernel() profiles the NEFF and stores
# exec_time_ns in LAST_EXEC_TIME_NS
TRACE = False
LAST_EXEC_TIME_NS = None
LAST_PROFILE = None

_KERNEL_CACHE = {}


def _install_ntff_hook():
    """antenv.axon_hooks shim so trace=True works under axon here."""
    import types

    try:
        from antenv.axon_hooks import get_axon_ntff_profile_hook  # noqa: F401
    except ImportError:
        import antenv

        m = types.ModuleType("antenv.axon_hooks")
        _store = {}
        m.set_axon_ntff_profile_hook = lambda h: _store.__setitem__("h", h)
        m.get_axon_ntff_profile_hook = lambda: _store.get("h")
        sys.modules["antenv.axon_hooks"] = m
        antenv.axon_hooks = m
    from antenv.axon_hooks import (
        get_axon_ntff_profile_hook,
        set_axon_ntff_profile_hook,
    )

    if get_axon_ntff_profile_hook() is None:
        from trn_agent_boot.trn_boot import _ntff_profile_via_ctypes

        set_axon_ntff_profile_hook(
            _ntff_profile_via_ctypes("/opt/axon/libaxon_pjrt.so")
        )
    from concourse import bass_utils

    bass_utils.upload_artifacts = lambda tmpdir: f"local://{tmpdir}"


def _chunks(total, step=512):
    out = []
    s = 0
    while s < total:
        out.append((s, min(step, total - s)))
        s += step
    return out


def _build(caps, TS):
    """Build the SPMD Bass kernel.

    DRAM params per core (all host-packed SBUF images; partition dim first):
      xb   [128, 4*NTOK] bf16   dispatched tokens; [p, t*NTOK+n] = x[t*128+p, n]
      xs   [128, 4*TS]   bf16   this core's shared-expert tokens, same layout
      w13  [EL, 128, 8192] f8e3  per expert: cols 0:4096 = 32*w1 image
                                 ([p, t*I+i] = w1[t*128+p, i]), 4096:8192 = 8*w3
      w2   [EL, 128, 4096] f8e4  DoubleRow image: [p, ((pr*2+x)*4+m)*128+c]
                                 = 32*w2[(2*pr+x)*128+p, m*128+c]
      wsh  [128, 12288]  bf16   ws1 img | ws3 img | ws2 img (plain layouts)
    Outputs:
      yexp [128, 4*NTOK] bf16   = 256 * y, same image layout as xb
      ysh  [128, 4*TS]   bf16   shared output image
    """
    NTOK = int(sum(caps))
    offs = [0]
    for c in caps:
        offs.append(offs[-1] + int(c))
    nc = bacc.Bacc("TRN2", target_bir_lowering=False)

    xb = nc.declare_dram_parameter("xb", [128, 4 * NTOK], BF16, isOutput=False)
    xq = nc.declare_dram_parameter("xq", [128, 4 * NTOK], F8E4, isOutput=False)
    xs = nc.declare_dram_parameter("xs", [128, 4 * TS], BF16, isOutput=False)
    w13a = nc.declare_dram_parameter("w13a", [EL, 128, 5120], BF16, isOutput=False)
    w13b = nc.declare_dram_parameter("w13b", [EL, 128, 3072], F8E4, isOutput=False)
    w2 = nc.declare_dram_parameter("w2", [EL, 128, 4096], F8E4, isOutput=False)
    wsh = nc.declare_dram_parameter("wsh", [128, 12288], BF16, isOutput=False)
    yexp = nc.declare_dram_parameter("yexp", [128, 4 * NTOK], BF16, isOutput=True)
    ysh = nc.declare_dram_parameter("ysh", [128, 4 * TS], BF16, isOutput=True)

    Silu = mybir.ActivationFunctionType.Silu
    Copy = mybir.ActivationFunctionType.Copy
    DR = mybir.MatmulPerfMode.DoubleRow

    with tile.TileContext(nc) as tc:
        with (
            tc.tile_pool(name="xpool", bufs=1) as xpool,
            tc.tile_pool(name="xbpool", bufs=3) as xbpool,
            tc.tile_pool(name="xqpool", bufs=3) as xqpool,
            tc.tile_pool(name="wshpool", bufs=1) as wshpool,
            tc.tile_pool(name="w13pool", bufs=4) as w13pool,
            tc.tile_pool(name="w13bpool", bufs=4) as w13bpool,
            tc.tile_pool(name="w2pool", bufs=4) as w2pool,
            tc.tile_pool(name="hpool", bufs=3) as hpool,
            tc.tile_pool(name="hspool", bufs=1) as hspool,
            tc.tile_pool(name="silpool", bufs=3) as silpool,
            tc.tile_pool(name="ypool", bufs=2) as ypool,
            tc.tile_pool(name="ps", bufs=3, space="PSUM") as pspool,
            tc.tile_pool(name="psy", bufs=2, space="PSUM") as psypool,
        ):
            # ---- input DMAs ----
            # ALL inputs go on the sync HWDGE ring, enqueued in exact
            # consumption order: a single queue row's descriptors drain FIFO
            # across all 16 SDMA engines at full HBM bandwidth, so the first
            # matmul's dependencies never compete with bulk prefetch (the
            # SDMA engines round-robin *between* rows, which would slow the
            # critical transfer ~6x if everything were spread across rows).
            w_tiles = {}
            x_tiles = {}

            def prefetch(e):
                cap = int(caps[e])
                xbe = xbpool.tile([128, 4, cap], BF16, tag="xbe", name="xbe")
                nc.sync.dma_start(
                    xbe[:],
                    xb[:, 4 * offs[e] : 4 * offs[e] + 4 * cap].rearrange(
                        "p (t c) -> p t c", t=4
                    ),
                )
                t13 = w13pool.tile([128, 5120], BF16, tag="w13a")
                nc.sync.dma_start(t13[:, 0:2560], w13a[e][:, 0:2560])
                nc.sync.dma_start(t13[:, 2560:5120], w13a[e][:, 2560:5120])
                xqe = xqpool.tile([128, 4, cap], F8E4, tag="xqe", name="xqe")
                nc.sync.dma_start(
                    xqe[:],
                    xq[:, 4 * offs[e] : 4 * offs[e] + 4 * cap].rearrange(
                        "p (t c) -> p t c", t=4
                    ),
                )
                t13b = w13bpool.tile([128, 3072], F8E4, tag="w13b")
                nc.sync.dma_start(t13b[:], w13b[e])
                t2 = w2pool.tile([128, 4096], F8E4, tag="w2")
                nc.sync.dma_start(t2[:], w2[e])
                w_tiles[e] = (t13, t13b, t2)
                x_tiles[e] = (xbe, xqe)

            # ring: expert0's transfers lead (first matmul waits only on
            # xbe0 + the 0.37MB w1 part), then shared-expert inputs, then
            # steady-state per-expert prefetch two experts ahead.
            prefetch(0)
            xssb = xpool.tile([128, 4 * TS], BF16, tag="xs")
            nc.sync.dma_start(xssb[:], xs[:])
            wshsb = wshpool.tile([128, 12288], BF16, tag="wsh")
            nc.sync.dma_start(wshsb[:, 0:8192], wsh[:, 0:8192])
            prefetch(1)
            nc.sync.dma_start(wshsb[:, 8192:12288], wsh[:, 8192:12288])
            prefetch(2)

            xs_r = xssb[:].rearrange("p (t n) -> p t n", n=TS)

            def stage1_shared(c0, cn):
                """bf16 SwiGLU stage1 of the shared expert; returns h tiles."""
                hs = []
                for j in range(8):
                    ps1 = pspool.tile([128, cn], F32, tag="ps1")
                    ps3 = pspool.tile([128, cn], F32, tag="ps3")
                    for t in range(4):
                        rhs = xs_r[:, t, c0 : c0 + cn]
                        nc.tensor.matmul(
                            ps1[:],
                            wshsb[:, t * I + j * 128 : t * I + (j + 1) * 128],
                            rhs,
                            start=(t == 0),
                            stop=(t == 3),
                        )
                    for t in range(4):
                        rhs = xs_r[:, t, c0 : c0 + cn]
                        nc.tensor.matmul(
                            ps3[:],
                            wshsb[:, 4096 + t * I + j * 128 : 4096 + t * I + (j + 1) * 128],
                            rhs,
                            start=(t == 0),
                            stop=(t == 3),
                        )
                    sil = silpool.tile([128, cn], F32, tag="sil")
                    nc.scalar.activation(sil[:], ps1[:], Silu)
                    h_j = hspool.tile([128, cn], BF16, tag=f"hs{j}")
                    nc.vector.tensor_mul(h_j[:], sil[:], ps3[:])
                    hs.append(h_j)

                def stage2():
                    yss = ypool.tile([128, 4 * cn], BF16, tag="yss")
                    for m in range(4):
                        psy = psypool.tile([128, cn], F32, tag="psy")
                        for t2 in range(8):
                            nc.tensor.matmul(
                                psy[:],
                                wshsb[:, 8192 + t2 * D + m * 128 : 8192 + t2 * D + (m + 1) * 128],
                                hs[t2][:],
                                start=(t2 == 0),
                                stop=(t2 == 7),
                            )
                        nc.scalar.activation(yss[:, m * cn : (m + 1) * cn], psy[:], Copy)
                    nc.scalar.dma_start(
                        ysh.rearrange("p (t n) -> p t n", n=TS)[:, :, c0 : c0 + cn],
                        yss[:].rearrange("p (t n) -> p t n", n=cn),
                    )

                return stage2

            def stage1_expert(le, c0, cn, tail=False):
                """fp8-weight stage1; h tiles land as 8*h in e4m3 pairs."""
                w13sb, w13bsb, w2sb = w_tiles[le]
                xbe, xqe = x_tiles[le]
                col0 = offs[le] + c0
                hp = []
                for pr in range(4):
                    hp.append(
                        hpool.tile(
                            [128, 2, cn], F8E4, tag=f"h{pr}", name=f"h{pr}"
                        )
                    )
                for j in range(8):
                    ps1 = pspool.tile([128, cn], F32, tag="ps1")
                    ps3 = pspool.tile([128, cn], F32, tag="ps3")
                    if j < 5:
                        for t in range(4):
                            rhs = xbe[:, t, c0 : c0 + cn]
                            nc.tensor.matmul(
                                ps1[:],
                                w13sb[:, j * 512 + t * 128 : j * 512 + (t + 1) * 128],
                                rhs,
                                start=(t == 0),
                                stop=(t == 3),
                            )
                        for t in range(4):
                            rhs = xbe[:, t, c0 : c0 + cn]
                            nc.tensor.matmul(
                                ps3[:],
                                w13sb[:, 2560 + j * 512 + t * 128 : 2560 + j * 512 + (t + 1) * 128],
                                rhs,
                                start=(t == 0),
                                stop=(t == 3),
                            )
                    else:
                        jj = j - 5
                        for u in range(2):
                            rhs = xqe[:, 2 * u : 2 * u + 2, c0 : c0 + cn]
                            nc.tensor.matmul(
                                ps1[:],
                                w13bsb[
                                    :, jj * 512 + u * 256 : jj * 512 + (u + 1) * 256
                                ].rearrange("p (v c) -> p v c", v=2),
                                rhs,
                                start=(u == 0),
                                stop=(u == 1),
                                perf_mode=DR,
                            )
                        for u in range(2):
                            rhs = xqe[:, 2 * u : 2 * u + 2, c0 : c0 + cn]
                            nc.tensor.matmul(
                                ps3[:],
                                w13bsb[
                                    :,
                                    1536 + jj * 512 + u * 256 : 1536 + jj * 512 + (u + 1) * 256,
                                ].rearrange("p (v c) -> p v c", v=2),
                                rhs,
                                start=(u == 0),
                                stop=(u == 1),
                                perf_mode=DR,
                            )
                    sil = silpool.tile([128, cn], F32, tag="sil")
                    # ps1 = 32*a -> silu(a)
                    nc.scalar.activation(sil[:], ps1[:], Silu, scale=1.0 / S1_W1_SCALE)
                    # ps3 = 8*b -> h tile = 8*silu(a)*b = 8*h  (|8h| < 240)
                    nc.vector.tensor_mul(hp[j // 2][:, j % 2, :], sil[:], ps3[:])

                w2_r = w2sb[:].rearrange("p (pr x m c) -> p pr x m c", pr=4, x=2, m=4)

                def stage2():
                    ysb = ypool.tile([128, 4 * cn], BF16, tag="ysb")
                    for m in range(4):
                        psy = psypool.tile([128, cn], F32, tag="psy")
                        for pr in range(4):
                            nc.tensor.matmul(
                                psy[:],
                                w2_r[:, pr, :, m, :],
                                hp[pr][:],
                                start=(pr == 0),
                                stop=(pr == 3),
                                perf_mode=DR,
                            )
                        if tail:
                            nc.vector.tensor_copy(
                                ysb[:, m * cn : (m + 1) * cn], psy[:]
                            )
                            # drain each m-tile immediately so the final
                            # store overlaps the remaining matmuls
                            nc.sync.dma_start(
                                yexp[:, m * NTOK + col0 : m * NTOK + col0 + cn],
                                ysb[:, m * cn : (m + 1) * cn],
                            )
                        else:
                            nc.scalar.activation(
                                ysb[:, m * cn : (m + 1) * cn], psy[:], Copy
                            )
                    if not tail:
                        nc.scalar.dma_start(
                            yexp.rearrange("p (t n) -> p t n", n=NTOK)[
                                :, :, col0 : col0 + cn
                            ],
                            ysb[:].rearrange("p (t n) -> p t n", n=cn),
                        )

                return stage2

            # ---- emission: expert0 first (its weights+tokens lead the DMA
            # ring), shared second (fills the gap while expert x/w stream),
            # then experts 1..7, software-pipelined (drain one stage2 per
            # stage1 so the PE always has independent matmul work while
            # ACT/DVE finish the current stage1's h tiles). ----
            pending = []
            for c0, cn in _chunks(int(caps[0])):
                pending.append(stage1_expert(0, c0, cn))
            for c0, cn in _chunks(TS):
                pending.append(stage1_shared(c0, cn))

            for le in range(1, EL):
                if le + 2 < EL:
                    prefetch(le + 2)
                last = le == EL - 1
                for c0, cn in _chunks(int(caps[le])):
                    s2 = stage1_expert(le, c0, cn, tail=last)
                    if pending:
                        pending.pop(0)()
                    pending.append(s2)
            while pending:
                pending.pop(0)()

    nc.compile()
    return nc


def _softmax(x):
    m = x.max(axis=-1, keepdims=True)
    e = np.exp(x - m)
    return e / e.sum(axis=-1, keepdims=True)


def _img_dxi(w, scale, dt):
    """[..., D_like, F] row-major -> [..., 128, (D_like/128)*F] SBUF image."""
    *lead, d, f = w.shape
    nt = d // 128
    v = np.clip(w * scale, -_dt_max(dt), _dt_max(dt)).astype(dt)
    v = v.reshape(*lead, nt, 128, f)
    v = np.moveaxis(v, -3, -2)  # [..., 128, nt, f]
    return np.ascontiguousarray(v.reshape(*lead, 128, nt * f))


def _img_jmajor(w, scale, dt):
    """[E, D, I] -> image [E, 128, 8, 512], [p, j, t*128+c] =
    scale*w[t*128+p, j*128+c]."""
    v = np.clip(w * scale, -_dt_max(dt), _dt_max(dt)).astype(dt)
    v = v.reshape(E, 4, 128, 8, 128)           # [e, t, p, j, c]
    v = v.transpose(0, 2, 3, 1, 4)             # [e, p, j, t, c]
    return np.ascontiguousarray(v.reshape(E, 128, 8, 512))


def _dt_max(dt):
    if dt is NP_E3M4:
        return 15.5
    if dt is NP_E4M3:
        return 240.0
    return np.finfo(np.float32).max


def kernel(x, gate_w, adaptive_bias, w1, w3, w2, ws1, ws3, ws2):
    global LAST_EXEC_TIME_NS, LAST_PROFILE

    x = np.asarray(x, dtype=np.float32)
    gate_w = np.asarray(gate_w, dtype=np.float32)
    adaptive_bias = np.asarray(adaptive_bias, dtype=np.float32)
    w1 = np.asarray(w1, dtype=np.float32)
    w3 = np.asarray(w3, dtype=np.float32)
    w2 = np.asarray(w2, dtype=np.float32)
    ws1 = np.asarray(ws1, dtype=np.float32)
    ws3 = np.asarray(ws3, dtype=np.float32)
    ws2 = np.asarray(ws2, dtype=np.float32)

    B, S, _ = x.shape
    T = B * S
    xf = x.reshape(T, D)

    # ---- gating (host, fp32, mirrors reference semantics) ----
    scores = xf @ gate_w.T + adaptive_bias
    probs = _softmax(scores)
    # jax.lax.top_k == stable descending sort, lower index wins ties
    topi = np.argsort(-probs, axis=-1, kind="stable")[:, :K].astype(np.int32)
    topw = np.take_along_axis(probs, topi, axis=-1)
    topw = topw / (topw.sum(axis=-1, keepdims=True) + 1e-8)

    flat_e = topi.reshape(-1)
    flat_w = topw.reshape(-1).astype(np.float32)
    flat_t = np.repeat(np.arange(T), K)

    order = np.argsort(flat_e, kind="stable")
    counts = np.bincount(flat_e, minlength=E)
    offsets = np.cumsum(counts) - counts
    slot_sorted = np.arange(T * K) - offsets[flat_e[order]]
    slot = np.empty(T * K, np.int64)
    slot[order] = slot_sorted
    valid = slot < CAP
    eff_counts = np.minimum(counts, CAP)

    # Assign experts to (core, slot) by load rank: slot s holds the experts
    # ranked [s*NCORES, (s+1)*NCORES), one per core, so every core has the
    # same per-slot capacity with minimal padding.
    perm = np.argsort(-eff_counts, kind="stable")        # expert ids by load desc
    rank = np.empty(E, np.int64)
    rank[perm] = np.arange(E)
    core_of = rank % NCORES
    slot_of = rank // NCORES
    caps = tuple(
        max(4, (int(eff_counts[perm[s * NCORES]]) + 3) // 4 * 4)
        for s in range(EL)
    )
    offs = np.concatenate([[0], np.cumsum(caps)])
    assert T % NCORES == 0
    TS = T // NCORES

    key = (caps, TS)
    if key not in _KERNEL_CACHE:
        _KERNEL_CACHE[key] = _build(caps, TS)
    nc = _KERNEL_CACHE[key]

    # ---- pack per-core inputs (exact SBUF images) ----
    xb16 = xf.astype(NP_BF16)

    w1i = _img_jmajor(w1, S1_W1_SCALE, NP_BF16)   # [E, 128, 8, 512]
    w3i = _img_jmajor(w3, S3_SCALE, NP_BF16)
    w1b = _img_jmajor(w1, S1_W1_SCALE, NP_E4M3)
    w3b = _img_jmajor(w3, S3_SCALE, NP_E4M3)
    w13a_all = np.ascontiguousarray(
        np.concatenate(
            [w1i[:, :, :5].reshape(E, 128, 2560), w3i[:, :, :5].reshape(E, 128, 2560)],
            axis=-1,
        )
    )  # [E, 128, 5120] bf16, j0-4 (scaled by 32/8, exact in bf16)
    w13b_all = np.ascontiguousarray(
        np.concatenate(
            [w1b[:, :, 5:].reshape(E, 128, 1536), w3b[:, :, 5:].reshape(E, 128, 1536)],
            axis=-1,
        )
    )  # [E, 128, 3072] e4m3, j5-7 (DoubleRow stage1)

    # w2 DoubleRow image: [E,I,D] -> [E, p, pr, x, m, c] -> [E, 128, 4096]
    w2q = np.clip(w2 * W2_SCALE, -240.0, 240.0).astype(NP_E4M3)
    w2q = w2q.reshape(E, 4, 2, 128, 4, 128)          # I = (pr, x, p); D = (m, c)
    w2q = np.ascontiguousarray(
        w2q.transpose(0, 3, 1, 2, 4, 5).reshape(E, 128, 4096)
    )

    wsh_img = np.concatenate(
        [
            _img_dxi(ws1, 1.0, NP_BF16),
            _img_dxi(ws3, 1.0, NP_BF16),
            _img_dxi(ws2, 1.0, NP_BF16),
        ],
        axis=-1,
    )  # [128, 12288]

    NTOK = int(sum(caps))
    v_idx = np.where(valid)[0]
    v_e = flat_e[v_idx]
    v_t = flat_t[v_idx]
    v_slot = slot[v_idx]
    v_core = core_of[v_e]
    v_col = offs[slot_of[v_e]] + v_slot  # column in that core's dispatch buffer

    in_maps = []
    for c in range(NCORES):
        m = v_core == c
        xbuf_c = np.zeros((NTOK, D), dtype=NP_BF16)
        xbuf_c[v_col[m]] = xb16[v_t[m]]
        # per-expert blocked image: block e at cols [4*off_e, 4*off_e+4*cap_e),
        # [p, 4*off_e + t*cap_e + n] = x[t*128+p, off_e+n]
        ximg = np.empty((128, 4 * NTOK), dtype=NP_BF16)
        for s in range(EL):
            off, cap = int(offs[s]), int(caps[s])
            blk = xbuf_c[off : off + cap].T  # [D, cap]
            ximg[:, 4 * off : 4 * off + 4 * cap] = (
                blk.reshape(4, 128, cap).transpose(1, 0, 2).reshape(128, 4 * cap)
            )
        xqimg = ximg.astype(np.float32).astype(NP_E4M3)
        xs_c = xb16[c * TS : (c + 1) * TS]
        xsimg = np.ascontiguousarray(
            xs_c.T.reshape(4, 128, TS).transpose(1, 0, 2).reshape(128, 4 * TS)
        )
        experts_c = perm[np.arange(EL) * NCORES + c]  # slot s -> expert id
        in_maps.append(
            {
                "xb": ximg,
                "xq": xqimg,
                "xs": xsimg,
                "w13a": np.ascontiguousarray(w13a_all[experts_c]),
                "w13b": np.ascontiguousarray(w13b_all[experts_c]),
                "w2": np.ascontiguousarray(w2q[experts_c]),
                "wsh": wsh_img,
            }
        )

    # ---- run on 8 cores ----
    if TRACE:
        _install_ntff_hook()
    res = run_bass_kernel_spmd(
        nc, in_maps, core_ids=list(range(NCORES)), trace=TRACE
    )
    LAST_EXEC_TIME_NS = res.exec_time_ns
    LAST_PROFILE = res

    # yexp per core: [128, 4*NTOK] bf16 image of 256*y ; ysh: [128, 4*TS]
    yexp = np.stack(
        [
            res.results[c]["yexp"]
            .astype(np.float32)
            .reshape(128, 4, NTOK)
            .transpose(1, 0, 2)
            .reshape(D, NTOK)
            for c in range(NCORES)
        ]
    )
    yshs = np.stack(
        [
            res.results[c]["ysh"]
            .astype(np.float32)
            .reshape(128, 4, TS)
            .transpose(1, 0, 2)
            .reshape(D, TS)
            for c in range(NCORES)
        ]
    )

    # ---- combine on host (1/256 descale folded into the weights) ----
    pair_y = np.zeros((T * K, D), np.float32)
    pair_y[v_idx] = yexp[v_core, :, v_col]  # gather [n_valid, D]
    w_eff = flat_w * valid.astype(np.float32) * OUT_DESCALE
    out = (pair_y * w_eff[:, None]).reshape(T, K, D).sum(axis=1)

    shared = yshs.transpose(0, 2, 1).reshape(T, D)
    out = out + shared
    return out.reshape(B, S, D).astype(np.float32)
